# revision 3
# baseline (speedup 1.0000x reference)
"""Trainium2 Bass kernel for nn_AttentionMap (B=4, S=4096, D=256 full attention).

Sharding: 8 cores = 4 batches x 2 query-halves (data-parallel batch,
sequence-parallel query rows). No collectives: each core computes
out[b, h*2048:(h+1)*2048, :] from conv_local[b] and its conv_global slice.

Per-core algorithm (all matmuls contract over the partition dim):
  phase 0: load X=conv_local[b] [4096,256], G=conv_global slice [2048,256];
           PE-transpose into XT [256,4096], GT [256,2048] (d on partitions).
  phase 1: KT = Wk^T XT + bk  [256,4096]   (lhsT=Wk chunk, rhs=XT chunk)
           QT = Wq^T GT + bq  [256,2048]
           V  = X Wv + bv     [4096,257]   (lhsT=XT chunk, rhs=Wv chunk),
           with an appended ones-column (V[:,256]=1) so the PV matmul also
           produces the softmax denominator.
  phase 2: per q-tile of 512 query rows:
           S^T chunks [128s,512q] = KT_chunk^T @ QT_tile (PSUM, fp32 accum)
           expS = exp(S^T / sqrt(256))  (ACT, PSUM->SBUF)   [no max-sub;
             scores ~ N(0,1) so exp is safe in fp32]
           O_unnorm[128q,257] = sum_s expS_chunk^T @ V_chunk  (PSUM accum);
           col 256 = sum_s exp = softmax denominator.
           out = O_unnorm[:, :256] * reciprocal(O_unnorm[:, 256])  -> DMA out.
"""

import os
import sys
from contextlib import ExitStack

import numpy as np

for _p in ("/opt/trn_rl_repo",):
    if _p not in sys.path and os.path.isdir(_p):
        sys.path.insert(0, _p)

import concourse.bass as bass
import concourse.mybir as mybir
import concourse.tile as tile
from concourse import bacc
from concourse.bass_utils import run_bass_kernel_spmd
from concourse.masks import make_identity

B = 4
S = 4096          # kv sequence length (= full query length)
D = 256           # model dim = head dim
NCORES = 8
SQH = S // 2      # query rows per core (2048)
QT = 512          # query tile (moving free dim of the S^T matmuls)
NQT = SQH // QT   # 4
NSC = S // 128    # 32 kv chunks of 128
NDC = D // 128    # 2 d chunks of 128
F32 = mybir.dt.float32
F32R = mybir.dt.float32r
BF16 = mybir.dt.bfloat16

# "f32" (exact), "f32r" (fast fp32 PE mode), "bf16"
MM_MODE = os.environ.get("ATTN_MM_MODE", "f32")

_CACHED = {}


def _mm(ap):
    """Bitcast a fp32 SBUF AP for the PE when running in f32r mode."""
    if MM_MODE == "f32r":
        return ap.bitcast(F32R)
    return ap


def build_program():
    nc = bacc.Bacc("TRN2", target_bir_lowering=False, debug=False)

    x_d = nc.dram_tensor("x", [S, D], F32, kind="ExternalInput").ap()
    g_d = nc.dram_tensor("g", [SQH, D], F32, kind="ExternalInput").ap()
    wk_d = nc.dram_tensor("wk", [D, D], F32, kind="ExternalInput").ap()
    wq_d = nc.dram_tensor("wq", [D, D], F32, kind="ExternalInput").ap()
    wv_d = nc.dram_tensor("wv", [D, D], F32, kind="ExternalInput").ap()
    bk_d = nc.dram_tensor("bk", [D, 1], F32, kind="ExternalInput").ap()
    bq_d = nc.dram_tensor("bq", [D, 1], F32, kind="ExternalInput").ap()
    bv_d = nc.dram_tensor("bv", [1, D], F32, kind="ExternalInput").ap()
    out_d = nc.dram_tensor("out", [SQH, D], F32, kind="ExternalOutput").ap()

    sb_dt = BF16 if MM_MODE == "bf16" else F32

    with tile.TileContext(nc) as tc, ExitStack() as ctx:
        Copy = mybir.ActivationFunctionType.Copy
        Ident = mybir.ActivationFunctionType.Identity
        Exp = mybir.ActivationFunctionType.Exp

        consts = ctx.enter_context(tc.tile_pool(name="consts", bufs=1))
        big = ctx.enter_context(tc.tile_pool(name="big", bufs=1))

        ident = consts.tile([128, 128], sb_dt)
        make_identity(nc, ident[:])

        wk_sb = consts.tile([128, NDC, D], sb_dt)
        wq_sb = consts.tile([128, NDC, D], sb_dt)
        wv_sb = consts.tile([128, NDC, D], sb_dt)
        bk_sb = consts.tile([128, NDC, 1], F32)
        bq_sb = consts.tile([128, NDC, 1], F32)
        bv_sb1 = consts.tile([1, D], F32)
        ones1 = consts.tile([1, 128], sb_dt)
        bv_bc = consts.tile([128, D], F32)

        if MM_MODE == "bf16":
            wld = consts.tile([128, 3 * NDC, D], F32, tag="wld")
            for kc in range(NDC):
                nc.sync.dma_start(wld[:, 0 * NDC + kc, :], wk_d[kc * 128:(kc + 1) * 128, :])
                nc.sync.dma_start(wld[:, 1 * NDC + kc, :], wq_d[kc * 128:(kc + 1) * 128, :])
                nc.sync.dma_start(wld[:, 2 * NDC + kc, :], wv_d[kc * 128:(kc + 1) * 128, :])
            for kc in range(NDC):
                nc.vector.tensor_copy(wk_sb[:, kc, :], wld[:, 0 * NDC + kc, :])
                nc.vector.tensor_copy(wq_sb[:, kc, :], wld[:, 1 * NDC + kc, :])
                nc.vector.tensor_copy(wv_sb[:, kc, :], wld[:, 2 * NDC + kc, :])
        else:
            for kc in range(NDC):
                nc.sync.dma_start(wk_sb[:, kc, :], wk_d[kc * 128:(kc + 1) * 128, :])
                nc.sync.dma_start(wq_sb[:, kc, :], wq_d[kc * 128:(kc + 1) * 128, :])
                nc.sync.dma_start(wv_sb[:, kc, :], wv_d[kc * 128:(kc + 1) * 128, :])
        for kc in range(NDC):
            nc.sync.dma_start(bk_sb[:, kc, :], bk_d[kc * 128:(kc + 1) * 128, :])
            nc.sync.dma_start(bq_sb[:, kc, :], bq_d[kc * 128:(kc + 1) * 128, :])
        nc.sync.dma_start(bv_sb1[:], bv_d[:])
        nc.vector.memset(ones1[:], 1.0)

        # ---- phase 2 SBUF residents (allocated first so they survive) ----
        kt = big.tile([128, NDC, S], sb_dt)       # K^T  [d, s]
        qt_sb = big.tile([128, NDC, SQH], sb_dt)  # Q^T  [d, q]
        vt = big.tile([128, NSC, D + 1], sb_dt)   # V||1 [s, d+1]

        with ExitStack() as p01:
            ld = p01.enter_context(tc.tile_pool(name="ld", bufs=4))
            trp = p01.enter_context(tc.tile_pool(name="trp", bufs=3, space="PSUM"))
            xtgt = p01.enter_context(tc.tile_pool(name="xtgt", bufs=1))
            mmp = p01.enter_context(tc.tile_pool(name="mmp", bufs=3, space="PSUM"))

            # bv broadcast across partitions via a K=1 matmul
            if MM_MODE == "bf16":
                bv_cast = consts.tile([1, D], BF16, tag="bvc")
                nc.vector.tensor_copy(bv_cast[:], bv_sb1[:])
                bv_rhs = bv_cast
            else:
                bv_rhs = bv_sb1
            psb = mmp.tile([128, D], F32, tag="proj")
            nc.tensor.matmul(psb[:], _mm(ones1[:]), _mm(bv_rhs[:]), start=True, stop=True)
            nc.vector.tensor_copy(bv_bc[:], psb[:])

            xt = xtgt.tile([128, NDC, S], sb_dt)    # X^T [d, s]
            gt = xtgt.tile([128, NDC, SQH], sb_dt)  # G^T [d, q]

            # ---- phase 0: load + transpose X and G ----
            for t in range(NSC):
                xld = ld.tile([128, D], F32, tag="ld")
                nc.sync.dma_start(xld[:], x_d[t * 128:(t + 1) * 128, :])
                if MM_MODE == "bf16":
                    xldc = ld.tile([128, D], BF16, tag="ldc")
                    nc.vector.tensor_copy(xldc[:], xld[:])
                    xsrc = xldc
                else:
                    xsrc = xld
                for kc in range(NDC):
                    ps = trp.tile([128, 128], sb_dt, tag="tr")
                    nc.tensor.transpose(ps[:], xsrc[:, kc * 128:(kc + 1) * 128], ident[:])
                    eng = nc.scalar if (t + kc) % 2 == 0 else nc.vector
                    if eng is nc.scalar:
                        nc.scalar.activation(xt[:, kc, t * 128:(t + 1) * 128], ps[:], Copy)
                    else:
                        nc.vector.tensor_copy(xt[:, kc, t * 128:(t + 1) * 128], ps[:])
            for t in range(SQH // 128):
                gld = ld.tile([128, D], F32, tag="ld")
                nc.sync.dma_start(gld[:], g_d[t * 128:(t + 1) * 128, :])
                if MM_MODE == "bf16":
                    gldc = ld.tile([128, D], BF16, tag="ldc")
                    nc.vector.tensor_copy(gldc[:], gld[:])
                    gsrc = gldc
                else:
                    gsrc = gld
                for kc in range(NDC):
                    ps = trp.tile([128, 128], sb_dt, tag="tr")
                    nc.tensor.transpose(ps[:], gsrc[:, kc * 128:(kc + 1) * 128], ident[:])
                    eng = nc.scalar if (t + kc) % 2 == 0 else nc.vector
                    if eng is nc.scalar:
                        nc.scalar.activation(gt[:, kc, t * 128:(t + 1) * 128], ps[:], Copy)
                    else:
                        nc.vector.tensor_copy(gt[:, kc, t * 128:(t + 1) * 128], ps[:])

            # ---- phase 1: projections ----
            # KT[dc, s] = sum_kc Wk[kc,dc]^T @ XT[kc, s] + bk[dc]
            for dc in range(NDC):
                for nt in range(S // 512):
                    ps = mmp.tile([128, 512], F32, tag="proj")
                    for kc in range(NDC):
                        nc.tensor.matmul(
                            ps[:],
                            _mm(wk_sb[:, kc, dc * 128:(dc + 1) * 128]),
                            _mm(xt[:, kc, nt * 512:(nt + 1) * 512]),
                            start=(kc == 0), stop=(kc == NDC - 1),
                        )
                    nc.scalar.activation(kt[:, dc, nt * 512:(nt + 1) * 512], ps[:],
                                         Ident, bias=bk_sb[:, dc, :])
            for dc in range(NDC):
                for nt in range(SQH // 512):
                    ps = mmp.tile([128, 512], F32, tag="proj")
                    for kc in range(NDC):
                        nc.tensor.matmul(
                            ps[:],
                            _mm(wq_sb[:, kc, dc * 128:(dc + 1) * 128]),
                            _mm(gt[:, kc, nt * 512:(nt + 1) * 512]),
                            start=(kc == 0), stop=(kc == NDC - 1),
                        )
                    nc.scalar.activation(qt_sb[:, dc, nt * 512:(nt + 1) * 512], ps[:],
                                         Ident, bias=bq_sb[:, dc, :])
            # V[s, :256] = X @ Wv + bv ; V[s, 256] = 1
            for t in range(NSC):
                ps = mmp.tile([128, D], F32, tag="proj")
                for kc in range(NDC):
                    nc.tensor.matmul(
                        ps[:],
                        _mm(xt[:, kc, t * 128:(t + 1) * 128]),
                        _mm(wv_sb[:, kc, :]),
                        start=(kc == 0), stop=(kc == NDC - 1),
                    )
                nc.vector.tensor_add(vt[:, t, 0:D], ps[:], bv_bc[:])
            nc.vector.memset(vt[:, :, D], 1.0)

        # ---- phase 2: attention ----
        es_bufs = 2 if MM_MODE == "bf16" else 1
        esp = ctx.enter_context(tc.tile_pool(name="esp", bufs=es_bufs))
        stp = ctx.enter_context(tc.tile_pool(name="stp", bufs=4, space="PSUM"))
        pvp = ctx.enter_context(tc.tile_pool(name="pvp", bufs=2, space="PSUM"))
        osb_p = ctx.enter_context(tc.tile_pool(name="osb", bufs=4))

        inv_sqrt_d = 1.0 / float(np.sqrt(D))
        for qi in range(NQT):
            q0 = qi * QT
            es = esp.tile([128, NSC, QT], sb_dt, tag="es")
            for t in range(NSC):
                ps = stp.tile([128, QT], F32, tag="st")
                for kc in range(NDC):
                    nc.tensor.matmul(
                        ps[:],
                        _mm(kt[:, kc, t * 128:(t + 1) * 128]),
                        _mm(qt_sb[:, kc, q0:q0 + QT]),
                        start=(kc == 0), stop=(kc == NDC - 1),
                    )
                nc.scalar.activation(es[:, t, :], ps[:], Exp, scale=inv_sqrt_d)
            for qs in range(QT // 128):
                acc = pvp.tile([128, D + 1], F32, tag="acc")
                for t in range(NSC):
                    nc.tensor.matmul(
                        acc[:],
                        _mm(es[:, t, qs * 128:(qs + 1) * 128]),
                        _mm(vt[:, t, :]),
                        start=(t == 0), stop=(t == NSC - 1),
                    )
                osb = osb_p.tile([128, D], F32, tag="osb")
                rec = osb_p.tile([128, 1], F32, tag="rec")
                nc.vector.reciprocal(rec[:], acc[:, D:D + 1])
                nc.vector.tensor_scalar_mul(osb[:], acc[:, 0:D], rec[:])
                nc.sync.dma_start(
                    out_d[q0 + qs * 128:q0 + (qs + 1) * 128, :], osb[:]
                )

    nc.compile()
    return nc


def _get_program():
    if "nc" not in _CACHED:
        _CACHED["nc"] = build_program()
    return _CACHED["nc"]


def kernel(conv_local, conv_global, Wk, bk, Wq, bq, Wv, bv):
    nc = _get_program()
    conv_local = np.ascontiguousarray(np.asarray(conv_local, dtype=np.float32))
    conv_global = np.ascontiguousarray(np.asarray(conv_global, dtype=np.float32))
    wk = np.ascontiguousarray(np.asarray(Wk, dtype=np.float32))
    wq = np.ascontiguousarray(np.asarray(Wq, dtype=np.float32))
    wv = np.ascontiguousarray(np.asarray(Wv, dtype=np.float32))
    bk = np.ascontiguousarray(np.asarray(bk, dtype=np.float32).reshape(D, 1))
    bq = np.ascontiguousarray(np.asarray(bq, dtype=np.float32).reshape(D, 1))
    bv = np.ascontiguousarray(np.asarray(bv, dtype=np.float32).reshape(1, D))

    in_maps = []
    for c in range(NCORES):
        b, h = c // 2, c % 2
        in_maps.append({
            "x": conv_local[b],
            "g": np.ascontiguousarray(conv_global[b, h * SQH:(h + 1) * SQH]),
            "wk": wk, "wq": wq, "wv": wv,
            "bk": bk, "bq": bq, "bv": bv,
        })

    trace = bool(int(os.environ.get("ATTN_TRACE", "0")))
    res = run_bass_kernel_spmd(nc, in_maps, list(range(NCORES)), trace=trace)
    _CACHED["last_results"] = res

    out = np.empty((B, S, D), dtype=np.float32)
    for c in range(NCORES):
        b, h = c // 2, c % 2
        out[b, h * SQH:(h + 1) * SQH] = res.results[c]["out"]
    return out
